# revision 1
# baseline (speedup 1.0000x reference)
"""Block-circulant linear layer (y = x @ W^T + bias, W built from 64x64
circulant blocks) on 8 Trainium2 NeuronCores.

Math: per output block j, input block i: y[t,j] = sum_i circ(c[j,i]) @ x[t,i].
Via the convolution theorem this is, for each rfft bin k:
    Yhat[t,j,k] = sum_i Chat[j,i,k] * Xhat[t,i,k]   (complex)
i.e. 33 independent complex [64 x 64] matmuls over the block index, batched
over tokens. The host does the cheap O(T*F*logB) DFTs + layout packing; the
device does the dominant compute — the per-frequency complex matmuls — packed
as real [128x128] @ [128x512] matmuls.

Real/complex packing (per frequency k, contraction over rows r):
    rhs rows r:   [Xr_i (64) ; Xi_i (64)],  cols = tokens
    lhsT[i,    j] =  Cr[j,i]    lhsT[i,    64+j] = Ci[j,i]
    lhsT[64+i, j] = -Ci[j,i]    lhsT[64+i, 64+j] = Cr[j,i]
    out rows:     [Yr_j (64) ; Yi_j (64)]
Bins k=0 and k=32 are purely real (real input DFT), so they share one tile
(kt=0) with a block-diagonal lhsT; kt=1..31 carry bin k = kt.

Sharding: by frequency tile (4 kt per core), NOT by tokens — the per-core
weight slice is then 131KB instead of a replicated 1.05MB, and the kernel is
DMA-bound (fp16 in+out is ~8.5MB/core; HBM sustains ~430B/ns single-direction,
~300 mixed). Schedule: loads as 4 big chunks (8KB rows) on the SP queue whose
FIFO then sequences the even stores strictly after them; odd stores drain in
parallel on ACT. Compute pipelines over 8 subchunks (matmul -> DVE/ACT cast),
with a PE warmup chain that connects seamlessly to the real matmul stream
(the PE p-state decays on any idle gap).
"""

import numpy as np

_B = 64          # circulant block size
_NBLK = 64       # input/output blocks (4096/64)
_NK = 33         # rfft bins of a 64-point real signal
_NKT = 32        # packed frequency tiles (k0+k32 share tile 0)
_NCORES = 8
_KTC = _NKT // _NCORES   # 4 frequency tiles per core
_T = 4096        # tokens = 2*2048
_F = 4096

_GL = 4           # token chunks per core for LOADS (8KB rows: ~430 B/ns reads)
_TCL = _T // _GL  # 1024 tokens per load chunk
_GS = 8           # subchunks for compute/stores (4KB rows, fine store pipeline)
_TCS = _T // _GS  # 512 tokens per store subchunk

# matmul input precision: "fp32r" (fp32 bits, TF32-grade multiply) or "fp16"
# (half the input DMA bytes, ~4x coarser rounding). Accumulation is fp32 either way.
_IN_PREC = "fp16"
# device->host precision of the frequency-domain result (fp16 halves store bytes;
# values are O(10) so fp16 rounding is ~2e-4 relative)
_OUT_PREC = "fp16"
_NP_IN = {"fp32r": np.float32, "fp16": np.float16}

_CACHE = {}


def _build_cmat(c):
    """c: [J=64, I=64, B=64] float32 -> packed lhsT matrix [128, NKT*128]."""
    fc = np.fft.rfft(np.asarray(c, np.float32), axis=-1)  # [J, I, 33] complex64
    Cr, Ci = fc.real, fc.imag
    cm = np.zeros((_NKT, 128, 128), np.float32)  # [kt, row, col]
    cm[0, 0:64, 0:64] = Cr[:, :, 0].T
    cm[0, 64:128, 64:128] = Cr[:, :, 32].T
    for k in range(1, 32):
        cm[k, 0:64, 0:64] = Cr[:, :, k].T
        cm[k, 64:128, 0:64] = -Ci[:, :, k].T
        cm[k, 0:64, 64:128] = Ci[:, :, k].T
        cm[k, 64:128, 64:128] = Cr[:, :, k].T
    # device layout: [128 partitions, kt*128 + col]
    out = np.ascontiguousarray(cm.transpose(1, 0, 2)).reshape(128, _NKT * 128)
    return out.astype(_NP_IN[_IN_PREC])


def _build_xk(x):
    """x: [2, 2048, 4096] float32 -> packed rhs [NKT, 128, T]."""
    xb = np.asarray(x, np.float32).reshape(_T, _NBLK, _B)
    fx = np.fft.rfft(xb, axis=-1)            # [T, I, 33] complex64
    R = fx.real.transpose(2, 1, 0)           # [33, I, T]
    Im = fx.imag.transpose(2, 1, 0)
    XKf = np.empty((_NKT, 128, _T), np.float32)
    XKf[0, 0:64] = R[0]
    XKf[0, 64:128] = R[32]
    XKf[1:32, 0:64] = R[1:32]
    XKf[1:32, 64:128] = Im[1:32]
    return XKf


def _unpack_y(YKf, bias):
    """YKf: [NKT, 128, T] device output -> y [2, 2048, 4096] float32."""
    re = np.zeros((_NK, _NBLK, _T), np.float32)
    im = np.zeros((_NK, _NBLK, _T), np.float32)
    re[0] = YKf[0, 0:64]
    re[32] = YKf[0, 64:128]
    re[1:32] = YKf[1:32, 0:64]
    im[1:32] = YKf[1:32, 64:128]
    Yf = (re + 1j * im).transpose(2, 1, 0)   # [T, J, 33]
    yb = np.fft.irfft(Yf, n=_B, axis=-1).astype(np.float32)  # [T, J, B]
    y = yb.reshape(_T, _F) + np.asarray(bias, np.float32)
    return np.ascontiguousarray(y.reshape(2, _T // 2, _F))


def _build_device():
    import concourse.bacc as bacc
    import concourse.mybir as mybir
    import concourse.tile as tile

    f32 = mybir.dt.float32
    mmdt = {
        "fp32r": mybir.dt.float32r,
        "fp16": mybir.dt.float16,
    }[_IN_PREC]
    outdt = {"fp32": f32, "fp16": mybir.dt.float16}[_OUT_PREC]
    nc = bacc.Bacc("TRN2", target_bir_lowering=False, debug=False)
    # load layout: [g][partition][kt_local*TCL + t], 8KB/row; store layout:
    # [s][partition][kt_local*TCS + t'], 4KB/row. cm is fused as a prefix of
    # chunk 0's transfer (one 9.2KB-row DMA maximizes bytes/descriptor during
    # the DMA-engine engagement ramp).
    _CMW = _KTC * 128
    cmx0 = nc.dram_tensor(
        "cmx0", [128, _CMW + _KTC * _TCL], mmdt, kind="ExternalInput"
    )
    xk = nc.dram_tensor(
        "xk", [_GL - 1, 128, _KTC * _TCL], mmdt, kind="ExternalInput"
    )
    yk = nc.dram_tensor("yk", [_GS, 128, _KTC * _TCS], outdt, kind="ExternalOutput")

    with tile.TileContext(nc) as tc:
        with (
            tc.tile_pool(name="cpool", bufs=1) as cpool,
            tc.tile_pool(name="xpool", bufs=1) as xpool,
            tc.tile_pool(name="ypool", bufs=1) as ypool,
            tc.tile_pool(name="pp", bufs=3, space="PSUM") as pp,
            tc.tile_pool(name="wpp", bufs=1, space="PSUM") as wpp,
        ):
            # all loads issued upfront on the SP ring; cm+x0 arrive fused.
            # Distinct buffers so no load waits on anything.
            cx = cpool.tile(
                [128, _CMW + _KTC * _TCL], mmdt, tag="cx", name="cx"
            )
            nc.sync.dma_start(out=cx[:], in_=cmx0[:, :])
            ct = cx
            xts = [(cx, _CMW)]
            for g in range(1, _GL):
                xt = xpool.tile([128, _KTC * _TCL], mmdt, tag=f"x{g}", name=f"x{g}")
                nc.sync.dma_start(out=xt[:], in_=xk[g - 1])
                xts.append((xt, 0))
            # PE warmup on zeroed tiles while the first loads are in flight:
            # ~3us of continuous matmul work ramps the PE p-state to 2.4GHz
            # before the real stream starts.
            wlhs = cpool.tile([128, 128], mmdt, tag="wlhs", name="wlhs")
            wrhs = cpool.tile([128, 512], mmdt, tag="wrhs", name="wrhs")
            nc.gpsimd.memset(wlhs[:], 0.0)
            nc.gpsimd.memset(wrhs[:], 0.0)
            wps = wpp.tile([128, 512], f32, name="wps")
            for _w in range(8):
                nc.tensor.matmul(
                    wps[:], lhsT=wlhs[:], rhs=wrhs[:], start=True, stop=True
                )
            # compute/store over 8 subchunks of 512 tokens; subchunk s reads
            # from load chunk s//2 at token offset (s%2)*512
            for s in range(_GS):
                xt, xb = xts[s * _TCS // _TCL]
                toff = xb + (s * _TCS) % _TCL
                # distinct buffer per subchunk: casts never wait store drains
                yt = ypool.tile([128, _KTC * _TCS], outdt, tag=f"y{s}", name=f"y{s}")
                if _TCS <= 512:
                    for h in range(_KTC // 2):
                        # 2-bank PSUM tile, two matmuls, one wide cast
                        ps = pp.tile([128, 2 * _TCS], f32)
                        for jj in range(2):
                            kt = h * 2 + jj
                            nc.tensor.matmul(
                                ps[:, jj * _TCS:(jj + 1) * _TCS],
                                lhsT=ct[:, kt * 128:(kt + 1) * 128],
                                rhs=xt[:, kt * _TCL + toff:kt * _TCL + toff + _TCS],
                                start=True,
                                stop=True,
                            )
                        dst = yt[:, h * 2 * _TCS:(h + 1) * 2 * _TCS]
                        # split casts across DVE and ACT
                        if h == 0:
                            nc.vector.tensor_copy(dst, ps[:])
                        else:
                            nc.scalar.copy(dst, ps[:])
                else:
                    # big subchunks: one 2-bank PSUM tile per kt, weight-
                    # stationary matmul pairs over token halves
                    for kt in range(_KTC):
                        ps = pp.tile([128, _TCS], f32)
                        for jj in range(_TCS // 512):
                            o = jj * 512
                            nc.tensor.matmul(
                                ps[:, o:o + 512],
                                lhsT=ct[:, kt * 128:(kt + 1) * 128],
                                rhs=xt[:, kt * _TCL + toff + o:
                                        kt * _TCL + toff + o + 512],
                                start=True,
                                stop=True,
                            )
                        dst = yt[:, kt * _TCS:(kt + 1) * _TCS]
                        if kt % 2 == 0:
                            nc.vector.tensor_copy(dst, ps[:])
                        else:
                            nc.scalar.copy(dst, ps[:])
                # even stores ride the SP queue (its FIFO sequences them after
                # all loads — mixed-direction HBM runs ~30% slower); odd
                # stores ride ACT, issued late enough that loads are done.
                # Two store queues drain the tail in parallel.
                if s % 2 == 0:
                    nc.sync.dma_start(out=yk[s], in_=yt[:])
                else:
                    nc.scalar.dma_start(out=yk[s], in_=yt[:])
    nc.compile()
    return nc


def _execute(in_maps, **kwargs):
    from concourse.bass_utils import run_bass_kernel_spmd

    if "nc" not in _CACHE:
        _CACHE["nc"] = _build_device()
    return run_bass_kernel_spmd(
        _CACHE["nc"], in_maps, core_ids=list(range(_NCORES)), **kwargs
    )


def _make_in_maps(x, c):
    XKf = _build_xk(x)
    cmd = _build_cmat(c)
    dt = _NP_IN[_IN_PREC]
    maps = []
    for m in range(_NCORES):
        s = XKf[m * _KTC:(m + 1) * _KTC]           # [KTC, 128, T]
        s = s.reshape(_KTC, 128, _GL, _TCL)        # [kt, p, g, t]
        xkm = np.ascontiguousarray(
            s.transpose(2, 1, 0, 3).reshape(_GL, 128, _KTC * _TCL)
        ).astype(dt)
        cmm = cmd[:, m * _KTC * 128:(m + 1) * _KTC * 128].astype(dt)
        # cm fused as a prefix of chunk 0's transfer
        cmx0 = np.ascontiguousarray(
            np.concatenate([cmm, xkm[0]], axis=1)
        )
        maps.append({"xk": np.ascontiguousarray(xkm[1:]), "cmx0": cmx0})
    return maps


def _gather_yk(results):
    """Per-core yk [GS, 128, KTC*TCS] -> full [NKT, 128, T]."""
    per_core = []
    for r in results:
        ykm = np.asarray(r["yk"]).reshape(_GS, 128, _KTC, _TCS)
        per_core.append(
            ykm.transpose(2, 1, 0, 3).reshape(_KTC, 128, _T)
        )
    return np.concatenate(per_core, axis=0)


def kernel(x, c, bias, **_kwargs):
    in_maps = _make_in_maps(x, c)
    bkr = _execute(in_maps)
    return _unpack_y(_gather_yk(bkr.results), bias)



# revision 2
# speedup vs baseline: 1.1189x; 1.1189x over previous
"""Block-circulant linear layer (y = x @ W^T + bias, W built from 64x64
circulant blocks) on 8 Trainium2 NeuronCores.

Math: per output block j, input block i: y[t,j] = sum_i circ(c[j,i]) @ x[t,i].
Via the convolution theorem this is, for each rfft bin k:
    Yhat[t,j,k] = sum_i Chat[j,i,k] * Xhat[t,i,k]   (complex)
i.e. 33 independent complex [64 x 64] matmuls over the block index, batched
over tokens. The host does the cheap O(T*F*logB) DFTs + layout packing; the
device does the dominant compute — the per-frequency complex matmuls — packed
as real [128x128] @ [128x512] matmuls.

Real/complex packing (per frequency k, contraction over rows r):
    rhs rows r:   [Xr_i (64) ; Xi_i (64)],  cols = tokens
    lhsT[i,    j] =  Cr[j,i]    lhsT[i,    64+j] = Ci[j,i]
    lhsT[64+i, j] = -Ci[j,i]    lhsT[64+i, 64+j] = Cr[j,i]
    out rows:     [Yr_j (64) ; Yi_j (64)]
Bins k=0 and k=32 are purely real (real input DFT), so they share one tile
(kt=0) with a block-diagonal lhsT; kt=1..31 carry bin k = kt.

Precision: X ships as fp8 e3m4 (1 byte) with a per-bin scale sx[k] =
absmax/15 folded into C (C' = Chat * sx, fp16).  The PE upconverts both
operands to fp22 and accumulates fp32, so the mixed fp16 x fp8 matmul is
exact given the quantized inputs; measured end-to-end max rel err ~1.4e-2
(gate 2e-2).  Y returns as fp16 (~3e-4 additional).

Sharding: by frequency tile (4 kt per core), NOT by tokens — the per-core
weight slice is 131KB instead of a replicated 1.05MB, and the kernel is
DMA-engine-bound: the 16 DMA engines sustain ~25 B/ns each (~410 B/ns/core
aggregate), so bytes moved is the whole game: 2.23MB in + 4.19MB out per
core.  Loads ride the SP queue whose FIFO then sequences the even stores
strictly after them (mixed-direction HBM runs ~30% slower); odd stores drain
in parallel on the ACT queue.  Compute pipelines over 8 subchunks
(matmul -> DVE/ACT cast); GpSimd has no PSUM port so only those two engines
can cast.
"""

import numpy as np
import ml_dtypes

_B = 64          # circulant block size
_NBLK = 64       # input/output blocks (4096/64)
_NK = 33         # rfft bins of a 64-point real signal
_NKT = 32        # packed frequency tiles (k0+k32 share tile 0)
_NCORES = 8
_KTC = _NKT // _NCORES   # 4 frequency tiles per core
_T = 4096        # tokens = 2*2048
_F = 4096

_GL = 4           # token chunks per core for LOADS (4KB rows at e3m4)
_TCL = _T // _GL  # 1024 tokens per load chunk
_GS = 8           # subchunks for compute/stores (4KB rows, fine store pipeline)
_TCS = _T // _GS  # 512 tokens per store subchunk

_E3 = ml_dtypes.float8_e3m4
_E3_TOP = 15.0    # scale X bins so absmax maps here (e3m4 max = 15.5)

_CACHE = {}


def _fold_scales(fc):
    """fc: [J, I, 33] complex64 -> (fc_scaled, sx[33]) with per-bin absmax
    scales to divide X by; the scale is multiplied into C."""
    return fc  # scaling handled in _pack_all


def _build_cmat(fc_s):
    """fc_s: [J, I, 33] complex64 (already bin-scaled) -> lhsT [128, NKT*128] fp16."""
    Cr, Ci = fc_s.real, fc_s.imag
    cm = np.zeros((_NKT, 128, 128), np.float32)  # [kt, row, col]
    cm[0, 0:64, 0:64] = Cr[:, :, 0].T
    cm[0, 64:128, 64:128] = Cr[:, :, 32].T
    for k in range(1, 32):
        cm[k, 0:64, 0:64] = Cr[:, :, k].T
        cm[k, 64:128, 0:64] = -Ci[:, :, k].T
        cm[k, 0:64, 64:128] = Ci[:, :, k].T
        cm[k, 64:128, 64:128] = Cr[:, :, k].T
    out = np.ascontiguousarray(cm.transpose(1, 0, 2)).reshape(128, _NKT * 128)
    return out.astype(np.float16)


def _pack_all(x, c):
    """-> (XKf [NKT,128,T] e3m4, cmat [128, NKT*128] fp16, sx[33])."""
    xb = np.asarray(x, np.float32).reshape(_T, _NBLK, _B)
    fx = np.fft.rfft(xb, axis=-1)            # [T, I, 33] complex64
    fc = np.fft.rfft(np.asarray(c, np.float32), axis=-1)  # [J, I, 33]
    R = np.ascontiguousarray(fx.real.transpose(2, 1, 0))   # [33, I, T]
    Im = np.ascontiguousarray(fx.imag.transpose(2, 1, 0))
    # per-bin scale: absmax over (t, i) of both components
    sx = np.maximum(np.abs(R).max(axis=(1, 2)), np.abs(Im).max(axis=(1, 2)))
    sx = np.where(sx > 0, sx, 1.0).astype(np.float32) / _E3_TOP   # [33]
    R /= sx[:, None, None]
    Im /= sx[:, None, None]
    XKf = np.empty((_NKT, 128, _T), _E3)
    XKf[0, 0:64] = R[0].astype(_E3)
    XKf[0, 64:128] = R[32].astype(_E3)
    XKf[1:32, 0:64] = R[1:32].astype(_E3)
    XKf[1:32, 64:128] = Im[1:32].astype(_E3)
    cmat = _build_cmat(fc * sx[None, None, :])
    return XKf, cmat


def _unpack_y(YKf, bias):
    """YKf: [NKT, 128, T] fp16 device output -> y [2, 2048, 4096] float32."""
    re = np.zeros((_NK, _NBLK, _T), np.float32)
    im = np.zeros((_NK, _NBLK, _T), np.float32)
    re[0] = YKf[0, 0:64]
    re[32] = YKf[0, 64:128]
    re[1:32] = YKf[1:32, 0:64]
    im[1:32] = YKf[1:32, 64:128]
    Yf = (re + 1j * im).transpose(2, 1, 0)   # [T, J, 33]
    yb = np.fft.irfft(Yf, n=_B, axis=-1).astype(np.float32)  # [T, J, B]
    y = yb.reshape(_T, _F) + np.asarray(bias, np.float32)
    return np.ascontiguousarray(y.reshape(2, _T // 2, _F))


def _build_device():
    import concourse.bacc as bacc
    import concourse.mybir as mybir
    import concourse.tile as tile

    f32 = mybir.dt.float32
    xdt = mybir.dt.float8e3
    cdt = mybir.dt.float16
    outdt = mybir.dt.float16
    nc = bacc.Bacc("TRN2", target_bir_lowering=False, debug=False)
    _CMW = _KTC * 128
    cw = nc.dram_tensor("cw", [128, _CMW], cdt, kind="ExternalInput")
    xk = nc.dram_tensor("xk", [_GL, 128, _KTC * _TCL], xdt, kind="ExternalInput")
    yk = nc.dram_tensor("yk", [_GS, 128, _KTC * _TCS], outdt, kind="ExternalOutput")

    with tile.TileContext(nc) as tc:
        with (
            tc.tile_pool(name="cpool", bufs=1) as cpool,
            tc.tile_pool(name="xpool", bufs=1) as xpool,
            tc.tile_pool(name="ypool", bufs=1) as ypool,
            tc.tile_pool(name="pp", bufs=3, space="PSUM") as pp,
        ):
            # all loads issued upfront on the SP ring; its FIFO sequences the
            # even stores strictly after them.  Distinct buffers so no load
            # waits on anything.
            ct = cpool.tile([128, _CMW], cdt, tag="cw", name="cw")
            nc.sync.dma_start(out=ct[:], in_=cw[:, :])
            xts = []
            for g in range(_GL):
                xt = xpool.tile([128, _KTC * _TCL], xdt, tag=f"x{g}", name=f"x{g}")
                nc.sync.dma_start(out=xt[:], in_=xk[g])
                xts.append(xt)
            # compute/store over 8 subchunks of 512 tokens; subchunk s reads
            # from load chunk s//2 at token offset (s%2)*512
            for s in range(_GS):
                xt = xts[s * _TCS // _TCL]
                toff = (s * _TCS) % _TCL
                # distinct buffer per subchunk: casts never wait store drains
                yt = ypool.tile([128, _KTC * _TCS], outdt, tag=f"y{s}", name=f"y{s}")
                for h in range(_KTC // 2):
                    # 2-bank PSUM tile, two matmuls, one wide cast
                    ps = pp.tile([128, 2 * _TCS], f32)
                    for jj in range(2):
                        kt = h * 2 + jj
                        nc.tensor.matmul(
                            ps[:, jj * _TCS:(jj + 1) * _TCS],
                            lhsT=ct[:, kt * 128:(kt + 1) * 128],
                            rhs=xt[:, kt * _TCL + toff:kt * _TCL + toff + _TCS],
                            start=True,
                            stop=True,
                        )
                    dst = yt[:, h * 2 * _TCS:(h + 1) * 2 * _TCS]
                    # split casts across DVE and ACT (only engines with a
                    # PSUM read port)
                    if h == 0:
                        nc.vector.tensor_copy(dst, ps[:])
                    else:
                        nc.scalar.copy(dst, ps[:])
                # even stores ride the SP queue (its FIFO sequences them after
                # all loads — mixed-direction HBM runs ~30% slower); odd
                # stores ride ACT, issued late enough that loads are done.
                # Two store queues drain the tail in parallel.
                if s % 2 == 0:
                    nc.sync.dma_start(out=yk[s], in_=yt[:])
                else:
                    nc.scalar.dma_start(out=yk[s], in_=yt[:])
    nc.compile()
    return nc


def _execute(in_maps, **kwargs):
    from concourse.bass_utils import run_bass_kernel_spmd

    if "nc" not in _CACHE:
        _CACHE["nc"] = _build_device()
    return run_bass_kernel_spmd(
        _CACHE["nc"], in_maps, core_ids=list(range(_NCORES)), **kwargs
    )


def _make_in_maps(x, c):
    XKf, cmd = _pack_all(x, c)
    maps = []
    for m in range(_NCORES):
        s = XKf[m * _KTC:(m + 1) * _KTC]           # [KTC, 128, T] e3m4
        s = s.reshape(_KTC, 128, _GL, _TCL)        # [kt, p, g, t]
        xkm = np.ascontiguousarray(
            s.transpose(2, 1, 0, 3).reshape(_GL, 128, _KTC * _TCL)
        )
        cmm = np.ascontiguousarray(cmd[:, m * _KTC * 128:(m + 1) * _KTC * 128])
        maps.append({"xk": xkm, "cw": cmm})
    return maps


def _gather_yk(results):
    """Per-core yk [GS, 128, KTC*TCS] -> full [NKT, 128, T]."""
    per_core = []
    for r in results:
        ykm = np.asarray(r["yk"]).reshape(_GS, 128, _KTC, _TCS)
        per_core.append(
            ykm.transpose(2, 1, 0, 3).reshape(_KTC, 128, _T)
        )
    return np.concatenate(per_core, axis=0)


def kernel(x, c, bias, **_kwargs):
    in_maps = _make_in_maps(x, c)
    bkr = _execute(in_maps)
    return _unpack_y(_gather_yk(bkr.results), bias)
